# revision 1
# baseline (speedup 1.0000x reference)
import numpy as np

# nn_GemmRS: input [WS=8, M=8192, K=512] x weight [WS=8, N=1024, K=512]
# -> per-rank partial GEMM [WS, M, N], then reduce-scatter over M:
# out[r] = sum_w partial[w, r*Ms:(r+1)*Ms, :], out shape [WS, Ms=1024, N].
# Strategy (per sharding_hint): one rank per NeuronCore — shard the leading
# WS axis across the 8 cores, compute the local GEMM on each core, and do
# the reduce-scatter over M on-device with lax.psum_scatter.

_compiled = None


def _get_compiled():
    global _compiled
    if _compiled is not None:
        return _compiled
    import jax
    import jax.numpy as jnp

    devs = jax.devices()
    if len(devs) < 8:
        raise RuntimeError("need 8 devices")

    @jax.pmap
    def _rank(x, w):
        # x: [M, K] local rank input; w: [N, K] local rank weight
        partial = jnp.einsum("mk,nk->mn", x, w)  # [M, N] partial sum
        # tiled reduce-scatter over rows: each rank keeps its Ms-row chunk
        return jax.lax.psum_scatter(partial, "x", scatter_dimension=0, tiled=True)

    # pmap default axis name is the mapped axis; bind explicitly instead
    _rank = jax.pmap(
        lambda x, w: jax.lax.psum_scatter(
            jnp.einsum("mk,nk->mn", x, w), "x", scatter_dimension=0, tiled=True
        ),
        axis_name="x",
        devices=devs[:8],
    )
    _compiled = _rank
    return _compiled


def kernel(input, weight):
    input = np.asarray(input)
    weight = np.asarray(weight)
    WS, M, K = input.shape
    N = weight.shape[1]
    Ms = M // WS
    try:
        fn = _get_compiled()
        out = fn(input.astype(np.float32), weight.astype(np.float32))
        out = np.asarray(out)  # [WS, Ms, N]
        if out.shape == (WS, Ms, N) and np.isfinite(out).all():
            return out.astype(np.float32)
    except Exception:
        pass
    # host fallback (always correct)
    partial = np.einsum(
        "wmk,wnk->wmn", input.astype(np.float32), weight.astype(np.float32)
    )
    return partial.reshape(WS, WS, Ms, N).sum(axis=0).astype(np.float32)

